# revision 24
# baseline (speedup 1.0000x reference)
"""DiceBCE + online-hard-negative-mining loss on 8 Trainium2 NeuronCores.

Strategy (memory-regime, single streaming pass over preds on device):
 - Host: exact positive-voxel stats (targs is only needed on host), n_hns,
   and a subsample-quantile estimate of the top-k threshold tau.
 - Device (per core, 1/8 shard of preds): u = bf16(relu(x - A)) for an anchor
   A just below tau; exact counts above 8 threshold grid points (DVE
   tensor_scalar + accum), sum softplus(u+A) / sigmoid(u+A) (ACT accum), plus
   full-tensor softplus/sigmoid totals and LUT probe values.
 - Host merge: exact-count threshold selection with fractional boundary-cell
   interpolation reconstructs sum softplus/sigmoid over the exact top-n_hns
   hard negatives; positives handled host-side in float64.
"""

import os
import numpy as np
import ml_dtypes

OHNM_RATIO = 30
DEFAULT_NEG_PERC = 0.1
EPS = 1e-10

NCORES = 8
P = 128
FREE = 16384          # per-core shard = [128, 16384] = 2,097,152 elements
TILE_F = 4096
NT = FREE // TILE_F   # 4 tiles
NC_CNT = 3            # count thresholds (first one is TINY -> count of u > 0)
NPRM = 32             # param tensor: [0]=A, [1..8]=count thresholds (u-space),
                      # [16..31] = probe values (x-space, sp/sg LUT readback)
TINY = np.float32(1e-35)

N_DVE_CNT = 3                  # count thresholds measured on DVE (is_ge)
N_ACT_CNT = NC_CNT - N_DVE_CNT  # count thresholds measured on ACT (Sign)

# output column layout
COL_C = 0                      # counts: NC_CNT * NT (DVE is_ge first, then ACT sign-sums)
COL_SP = COL_C + NC_CNT * NT   # sum sp2(m) = ln(1+exp(-m)): NT
COL_SG = COL_SP + NT           # sum sigmoid(m): NT
COL_M = COL_SG + NT            # sum m (exact f32): NT
COL_PSP = COL_M + NT           # probe sp2: NPRM
COL_PSG = COL_PSP + NPRM       # probe sigmoid: NPRM
NOUT = COL_PSG + NPRM

_CACHE = {}


def _np_softplus(x):
    x = np.asarray(x, np.float64)
    return np.maximum(x, 0) + np.log1p(np.exp(-np.abs(x)))


def _np_sigmoid(x):
    x = np.asarray(x, np.float64)
    return 0.5 * (1 + np.tanh(x / 2))


def build_nc(free=FREE, tile_f=TILE_F):
    """Build the Bass module (one NeuronCore program, run SPMD on 8 cores)."""
    from contextlib import ExitStack
    import concourse.bass as bass
    import concourse.tile as tile
    from concourse import bacc, mybir

    nt = free // tile_f
    f32 = mybir.dt.float32
    bf16 = mybir.dt.bfloat16
    Alu = mybir.AluOpType
    Act = mybir.ActivationFunctionType

    nc = bacc.Bacc(
        "TRN2",
        target_bir_lowering=False,
        debug=False,
        enable_asserts=False,
        num_devices=NCORES,
    )
    x_ap = nc.dram_tensor("x", (P, free), f32, kind="ExternalInput").ap()
    prm_ap = nc.dram_tensor("prm", (1, NPRM), f32, kind="ExternalInput").ap()
    nout = NC_CNT * nt + 3 * nt + 2 * NPRM
    out_ap = nc.dram_tensor("out", (P, nout), f32, kind="ExternalOutput").ap()

    act_chain = []  # enforce ACT program order so act-table reloads stay rare

    def chain(inst):
        if act_chain:
            bass._add_dep_helper(inst.ins, act_chain[-1].ins, sync=False,
                                 reason="act table phase order")
        act_chain.append(inst)
        return inst

    with tile.TileContext(nc) as tc, ExitStack() as ctx:
        const_pool = ctx.enter_context(tc.tile_pool(name="const", bufs=1))
        xpool = ctx.enter_context(tc.tile_pool(name="xin", bufs=2))
        mpool = ctx.enter_context(tc.tile_pool(name="m", bufs=3))
        upool = ctx.enter_context(tc.tile_pool(name="u", bufs=1))
        sapool = ctx.enter_context(tc.tile_pool(name="sa", bufs=1))
        jd_pool = ctx.enter_context(tc.tile_pool(name="junkd", bufs=2))
        ja_pool = ctx.enter_context(tc.tile_pool(name="junka", bufs=2))
        acc_pool = ctx.enter_context(tc.tile_pool(name="acc", bufs=1))

        # params -> all partitions
        prm_row = const_pool.tile([1, NPRM], f32, tag="prmrow")
        nc.sync.dma_start(prm_row[:], prm_ap[:, :])
        prm_sb = const_pool.tile([P, NPRM], f32, tag="prmsb")
        nc.gpsimd.partition_broadcast(prm_sb[:], prm_row[:])
        a_ap = prm_sb[:, 0:1]

        # one accumulator arena; subtile dep tracking keeps writes parallel
        nacc = (NC_CNT + 3) * nt
        arena = acc_pool.tile([P, nacc], f32, tag="arena")
        accC = [arena[:, i * nt:(i + 1) * nt] for i in range(NC_CNT)]
        accSP = arena[:, NC_CNT * nt:(NC_CNT + 1) * nt]
        accSG = arena[:, (NC_CNT + 1) * nt:(NC_CNT + 2) * nt]
        accM = arena[:, (NC_CNT + 2) * nt:(NC_CNT + 3) * nt]

        sas = []
        uts = []
        # DVE: per tile [m_t, u_t] hoisted before the previous tile's counts
        # so ACT's sigmoid chain is never starved; ACT: sigmoid/Ln pairwise
        # phases to overlap Ln work with DVE counts (act-table loads ~1.3us).
        def emit_counts(t):
            for i in range(N_DVE_CNT):
                jt = jd_pool.tile([P, tile_f], bf16, tag="junkd", name="junkd")
                nc.vector.tensor_scalar(
                    out=jt[:], in0=uts[t][:], scalar1=prm_sb[:, 1 + i:2 + i],
                    scalar2=None, op0=Alu.is_ge, op1=Alu.add,
                    accum_out=accC[i][:, t:t + 1],
                )

        def emit_ln(t):
            chain(nc.scalar.activation(
                out=sas[t][:], in_=sas[t][:], func=Act.Ln,
                accum_out=accSP[:, t:t + 1],
            ))

        for t in range(nt):
            xt = xpool.tile([P, tile_f], f32, tag="xt", name="xt")
            nc.sync.dma_start(xt[:], x_ap[:, bass.ts(t, tile_f)])

            mt = mpool.tile([P, tile_f], f32, tag="mt", name="mt")
            nc.vector.tensor_scalar(
                out=mt[:], in0=xt[:], scalar1=a_ap, scalar2=None, op0=Alu.max)

            # u = m - A = relu(x - A) (bf16 for cheap counts); the same op
            # accumulates sum(m - A) in f32; host adds N*A back in f64
            ut = upool.tile([P, tile_f], bf16, tag=f"ut{t}", name=f"ut{t}")
            nc.vector.tensor_scalar(
                out=ut[:], in0=mt[:], scalar1=a_ap, scalar2=None,
                op0=Alu.subtract, op1=Alu.add,
                accum_out=accM[:, t:t + 1])
            uts.append(ut)

            # sigmoid sum over m; output kept for sp2 = -ln(sigmoid(m))
            sa = sapool.tile([P, tile_f], f32, tag=f"sa{t}", name=f"sa{t}")
            chain(nc.scalar.activation(
                out=sa[:], in_=mt[:], func=Act.Sigmoid,
                accum_out=accSG[:, t:t + 1],
            ))
            sas.append(sa)

            if t == 1:
                emit_counts(0)
                emit_ln(0)
                emit_ln(1)
        for t in range(1, nt):
            emit_counts(t)

        # sigmoid probe while still on the sigmoid table
        psg = const_pool.tile([P, NPRM], f32, tag="psg")
        chain(nc.scalar.activation(out=psg[:], in_=prm_sb[:], func=Act.Sigmoid))

        for t in range(2, nt):
            emit_ln(t)

        # softplus-residual probe via the same sigmoid -> Ln chain
        psp = const_pool.tile([P, NPRM], f32, tag="psp")
        chain(nc.scalar.activation(out=psp[:], in_=psg[:], func=Act.Ln))

        # write outputs
        col = (NC_CNT + 3) * nt
        nc.sync.dma_start(out_ap[:, 0:col], arena[:])
        nc.sync.dma_start(out_ap[:, col:col + NPRM], psp[:])
        col += NPRM
        nc.sync.dma_start(out_ap[:, col:col + NPRM], psg[:])

    nc.compile()
    return nc


def _get_nc():
    if "nc" not in _CACHE:
        _CACHE["nc"] = build_nc()
    return _CACHE["nc"]


def _host_prepass(preds_flat, targs_flat):
    N = preds_flat.size
    pos_mask = targs_flat == 1
    n_pos = int(pos_mask.sum())
    pos_x = preds_flat[pos_mask]
    n_neg = N - n_pos
    if n_pos == 0:
        n_hns = int(DEFAULT_NEG_PERC * n_neg)
    else:
        n_hns = min(n_pos * OHNM_RATIO, n_neg)

    pos64 = pos_x.astype(np.float64)
    S_pos_sp = _np_softplus(pos64).sum()
    S_pos_sg = _np_sigmoid(pos64).sum()
    S_pos_x = pos64.sum()

    # threshold estimate from a subsample of negatives
    sub = preds_flat[::16]
    subn = sub[targs_flat[::16] == 0]
    if n_hns >= n_neg:
        # select-all-negatives: anchor far below the data so m = x exactly
        tau_hat, spread = -40.0, 1.0
    elif n_hns <= 0 or len(subn) < 100:
        tau_hat, spread = 0.0, 1.0
    else:
        q = n_hns / n_neg
        r = max(1, min(int(round(q * len(subn))), len(subn) - 1))
        part = np.partition(subn, len(subn) - r)
        tau_hat = float(part[len(subn) - r])
        h = 0.05
        dens = ((subn > tau_hat - h) & (subn < tau_hat + h)).sum() / (2 * h * len(subn))
        sig = np.sqrt(q * (1 - q) / len(subn)) / max(dens, 1e-9)
        spread = float(max(8 * sig, 0.005))
    return dict(N=N, n_pos=n_pos, n_neg=n_neg, n_hns=n_hns, pos_x=pos_x,
                S_pos_sp=S_pos_sp, S_pos_sg=S_pos_sg, S_pos_x=S_pos_x,
                tau_hat=tau_hat, spread=spread)


def _make_params(tau_hat, spread):
    """A (f32 anchor), u-space count thresholds cs, x-space bounds, prm tensor."""
    rel = np.linspace(-1.0, 1.0, NC_CNT - 1)
    A = np.float32(tau_hat - 1.5 * spread)
    grid = tau_hat + rel * spread
    cs = np.concatenate([[TINY], (grid - float(A)).astype(np.float32)]).astype(np.float32)
    assert len(cs) == NC_CNT
    # thresholds counted via ACT Sign must not tie with bf16 u values
    for i in range(N_DVE_CNT, NC_CNT):
        c = np.float32(cs[i])
        if np.float32(ml_dtypes.bfloat16(c)) == c:
            c = np.float32(c + abs(c) * 2.0 ** -10 + 1e-30)
        cs[i] = c
    bounds = np.concatenate([[float(A)], float(A) + cs[1:].astype(np.float64)])
    prm = np.zeros((1, NPRM), np.float32)
    prm[0, 0] = A
    prm[0, 1:1 + NC_CNT] = cs
    prm[0, 9:9 + (NC_CNT - N_DVE_CNT)] = -cs[N_DVE_CNT:]
    # probe slots: the bounds and cell midpoints (x-space)
    mids = 0.5 * (bounds[:-1] + bounds[1:])
    probes = np.concatenate([bounds, mids])  # 8 + 7 = 15 values
    prm[0, 16:16 + len(probes)] = probes.astype(np.float32)
    return A, cs, bounds, prm


def _get_runner():
    """Cached jitted SPMD runner (mirrors bass2jax.run_bass_via_pjrt, but the
    lowered/jitted callable is built once and reused across calls)."""
    if "runner" in _CACHE:
        return _CACHE["runner"]
    import jax
    import numpy as _np
    from jax.sharding import Mesh, PartitionSpec
    from jax.experimental.shard_map import shard_map
    from concourse import mybir
    from concourse.bass2jax import (_bass_exec_p, install_neuronx_cc_hook,
                                    partition_id_tensor)

    install_neuronx_cc_hook()
    nc = _get_nc()
    partition_name = (nc.partition_id_tensor.name
                      if nc.partition_id_tensor else None)

    in_names, out_names, out_avals, zero_outs = [], [], [], []
    for alloc in nc.m.functions[0].allocations:
        if not isinstance(alloc, mybir.MemoryLocationSet):
            continue
        name = alloc.memorylocations[0].name
        if alloc.kind == "ExternalInput":
            if name != partition_name:
                in_names.append(name)
        elif alloc.kind == "ExternalOutput":
            out_names.append(name)
            shape = tuple(alloc.tensor_shape)
            dtype = mybir.dt.np(alloc.dtype)
            out_avals.append(jax.core.ShapedArray(shape, dtype))
            zero_outs.append(_np.zeros(shape, dtype))
    n_params = len(in_names)
    n_outs = len(out_avals)
    all_names = in_names + out_names
    if partition_name is not None:
        all_names = all_names + [partition_name]

    def _body(*args):
        operands = list(args)
        if partition_name is not None:
            operands.append(partition_id_tensor())
        outs = _bass_exec_p.bind(
            *operands,
            out_avals=tuple(out_avals),
            in_names=tuple(all_names),
            out_names=tuple(out_names),
            lowering_input_output_aliases=(),
            sim_require_finite=True,
            sim_require_nnan=True,
            nc=nc,
        )
        return tuple(outs)

    devices = jax.devices()[:NCORES]
    mesh = Mesh(np.asarray(devices), ("core",))
    in_specs = (PartitionSpec("core"),) * (n_params + n_outs)
    out_specs = (PartitionSpec("core"),) * n_outs
    donate = tuple(range(n_params, n_params + n_outs))
    sharded = jax.jit(
        shard_map(_body, mesh=mesh, in_specs=in_specs, out_specs=out_specs,
                  check_rep=False),
        donate_argnums=donate, keep_unused=True,
    )
    _CACHE["runner"] = (sharded, in_names, out_names, zero_outs)
    return _CACHE["runner"]


def _run_device(shards, prm):
    """shards: [8, 128, FREE] f32. Returns list of 8 out arrays [P, NOUT]."""
    sharded, in_names, out_names, zero_outs = _get_runner()
    per_core = [{"x": shards[c], "prm": prm} for c in range(NCORES)]
    concat_in = [np.concatenate([per_core[c][n] for c in range(NCORES)], axis=0)
                 for n in in_names]
    concat_zeros = [np.zeros((NCORES * z.shape[0], *z.shape[1:]), z.dtype)
                    for z in zero_outs]
    out_arrs = sharded(*concat_in, *concat_zeros)
    res = []
    for c in range(NCORES):
        d = {}
        for i, name in enumerate(out_names):
            arr = np.asarray(out_arrs[i])
            rows = arr.shape[0] // NCORES
            d[name] = arr[c * rows:(c + 1) * rows]
        res.append(d)
    _CACHE["last_outs"] = res
    return [r["out"] for r in res]


def _merge(outs, ph, A, cs, bounds, prm):
    """Host-side merge of per-core outputs into the final scalar."""
    N, n_pos, n_hns = ph["N"], ph["n_pos"], ph["n_hns"]
    n_neg = ph["n_neg"]
    tot = np.zeros(NOUT, np.float64)
    for o in outs:
        tot += o.astype(np.float64).sum(axis=0)
    C = np.array([tot[COL_C + i * NT: COL_C + (i + 1) * NT].sum()
                  for i in range(NC_CNT)])
    # ACT-measured counts arrive as sum(sign(u - c)) = 2*C - N
    for i in range(N_DVE_CNT, NC_CNT):
        C[i] = (C[i] + N) / 2.0
    SP2 = -tot[COL_SP:COL_SP + NT].sum()  # sum ln(1+exp(-m)) = -sum ln(sigmoid)
    SG = tot[COL_SG:COL_SG + NT].sum()
    # COL_M holds sum(m - A); reconstruct sum(m) in f64
    SM = tot[COL_M:COL_M + NT].sum() + N * float(A)
    SP = SM + SP2                          # sum softplus(m)

    # device-LUT values at probes (mean across cores/partitions; identical data)
    denom_probe = NCORES * P
    psp = tot[COL_PSP:COL_PSP + NPRM] / denom_probe
    psg = tot[COL_PSG:COL_PSG + NPRM] / denom_probe
    probe_x = prm[0, 16:].astype(np.float64)
    # LUT correction: delta(v) = lut(v) - exact(v), interpolated at probes
    pv = probe_x[:15]
    order = np.argsort(pv)
    pv_s = pv[order]
    dsp_s = (-psp[16:31] - _np_softplus(-pv))[order]
    dsg_s = (psg[16:31] - _np_sigmoid(pv))[order]

    def lut_sp(v):
        # softplus(v) = v + sp2(v); device measures sp2 via exp/ln chain
        return float(v + _np_softplus(-np.asarray(v, np.float64))
                     + np.interp(v, pv_s, dsp_s))

    def lut_sg(v):
        return float(_np_sigmoid(v) + np.interp(v, pv_s, dsg_s))

    # subtract positives' contribution to device stats (host-exact simulation)
    pos32 = ph["pos_x"].astype(np.float32)
    Af = np.float32(A)
    upos = np.maximum(pos32 - Af, 0).astype(ml_dtypes.bfloat16).astype(np.float32)
    Cpos = np.array([(upos >= c).sum() for c in cs], np.float64)
    mpos = np.maximum(pos32, Af).astype(np.float64)      # f32 m path
    SPpos = (mpos + _np_softplus(-mpos) + np.interp(mpos, pv_s, dsp_s)).sum()
    SGpos = (_np_sigmoid(mpos) + np.interp(mpos, pv_s, dsg_s)).sum()

    Cn = C - Cpos
    SPn = SP - SPpos
    SGn = SG - SGpos
    Nn = N - n_pos

    if n_hns <= 0:
        sel_sp = 0.0
        sel_sg = 0.0
    else:
        k = float(n_hns)
        if k > Cn[0] or k < Cn[-1]:
            raise BracketMiss(Cn)
        sel_sp = SPn - (Nn - Cn[0]) * lut_sp(A)
        sel_sg = SGn - (Nn - Cn[0]) * lut_sg(A)
        j = 0
        while j + 1 < NC_CNT and Cn[j + 1] >= k:
            j += 1
        for i in range(j):
            pop = Cn[i] - Cn[i + 1]
            mid = 0.5 * (bounds[i] + bounds[i + 1])
            sel_sp -= pop * lut_sp(mid)
            sel_sg -= pop * lut_sg(mid)
        pop_j = Cn[j] - (Cn[j + 1] if j + 1 < NC_CNT else 0.0)
        excl = Cn[j] - k
        hi = bounds[j + 1] if j + 1 < NC_CNT else bounds[j] + 1.0
        if excl > 0 and pop_j > 0:
            f = excl / pop_j
            mid = bounds[j] + 0.5 * f * (hi - bounds[j])
            sel_sp -= excl * lut_sp(mid)
            sel_sg -= excl * lut_sg(mid)

    inter = ph["S_pos_sg"]
    denom = (sel_sg + ph["S_pos_sg"]) + n_pos
    dice = 1.0 - (2.0 * inter + EPS) / (denom + EPS)
    bce = (sel_sp + (ph["S_pos_sp"] - ph["S_pos_x"])) / (n_hns + n_pos)
    return np.float32(dice + bce)


class BracketMiss(RuntimeError):
    def __init__(self, counts):
        super().__init__(f"bracket miss: counts={counts}")
        self.counts = counts


def kernel(preds, targs):
    preds_flat = np.asarray(preds, np.float32).ravel()
    targs_flat = np.asarray(targs).ravel()
    ph = _host_prepass(preds_flat, targs_flat)

    shards = preds_flat.reshape(NCORES, P, FREE)

    tau_hat, spread = ph["tau_hat"], ph["spread"]
    for attempt in range(4):
        A, cs, bounds, prm = _make_params(tau_hat, spread)
        outs = _run_device(shards, prm)
        try:
            return _merge(outs, ph, A, cs, bounds, prm)
        except BracketMiss as bm:
            # widen and recenter using the measured counts, then relaunch
            Cn = bm.counts
            k = ph["n_hns"]
            if k > Cn[0]:
                tau_hat = float(A) - 2.0 * spread
            else:
                tau_hat = float(bounds[-1]) + 2.0 * spread
            spread *= 4.0
    raise RuntimeError("failed to bracket top-k threshold after 4 attempts")


if __name__ == "__main__":
    # quick self-test against numpy ground truth (no jax needed)
    rng = np.random.default_rng(0)
    preds = rng.standard_normal((1, 1, 256, 256, 256), np.float32)
    targs = (rng.random((1, 1, 256, 256, 256)) < 1e-3).astype(np.int32)
    out = kernel(preds, targs)
    print("kernel out:", out)


# revision 28
# speedup vs baseline: 1.0399x; 1.0399x over previous
"""DiceBCE + online-hard-negative-mining loss on 8 Trainium2 NeuronCores.

Strategy (memory-regime, single streaming pass over preds on device):
 - Host: exact positive-voxel stats (targs is only needed on host), n_hns,
   and a subsample-quantile estimate of the top-k threshold tau.
 - Device (per core, 1/8 shard of preds): u = bf16(relu(x - A)) for an anchor
   A just below tau; exact counts above 8 threshold grid points (DVE
   tensor_scalar + accum), sum softplus(u+A) / sigmoid(u+A) (ACT accum), plus
   full-tensor softplus/sigmoid totals and LUT probe values.
 - Host merge: exact-count threshold selection with fractional boundary-cell
   interpolation reconstructs sum softplus/sigmoid over the exact top-n_hns
   hard negatives; positives handled host-side in float64.
"""

import os
import numpy as np
import ml_dtypes

OHNM_RATIO = 30
DEFAULT_NEG_PERC = 0.1
EPS = 1e-10

NCORES = 8
P = 128
FREE = 16384          # per-core shard = [128, 16384] = 2,097,152 elements
TILE_F = 4096
TILES = [1024, 4096, 4096, 4096, 3072]   # per-tile free dims (sum = FREE)
NT = len(TILES)
# ACT chain plan: after sig_j (key), emit Ln passes for tiles in value
LN_PLAN = {1: [0, 1]}
NC_CNT = 2            # count thresholds (real grid points; no u>0 count)
NPRM = 32             # param tensor: [0]=A, [1..8]=count thresholds (u-space),
                      # [16..31] = probe values (x-space, sp/sg LUT readback)
TINY = np.float32(1e-35)

N_DVE_CNT = 2                  # count thresholds measured on DVE (is_ge)
N_ACT_CNT = NC_CNT - N_DVE_CNT  # count thresholds measured on ACT (Sign)

# output column layout
COL_C = 0                      # counts: NC_CNT * NT (DVE is_ge first, then ACT sign-sums)
COL_SP = COL_C + NC_CNT * NT   # sum sp2(m) = ln(1+exp(-m)): NT
COL_SG = COL_SP + NT           # sum sigmoid(m): NT
COL_M = COL_SG + NT            # sum m (exact f32): NT
COL_PSP = COL_M + NT           # probe sp2: NPRM
COL_PSG = COL_PSP + NPRM       # probe sigmoid: NPRM
NOUT = COL_PSG + NPRM

_CACHE = {}


def _np_softplus(x):
    x = np.asarray(x, np.float64)
    return np.maximum(x, 0) + np.log1p(np.exp(-np.abs(x)))


def _np_sigmoid(x):
    x = np.asarray(x, np.float64)
    return 0.5 * (1 + np.tanh(x / 2))


def build_nc(free=FREE, tile_f=None, tiles=None, ln_plan=None):
    """Build the Bass module (one NeuronCore program, run SPMD on 8 cores)."""
    from contextlib import ExitStack
    import concourse.bass as bass
    import concourse.tile as tile
    from concourse import bacc, mybir

    if tiles is None:
        tiles = [tile_f] * (free // tile_f) if tile_f else list(TILES)
    if ln_plan is None:
        ln_plan = LN_PLAN if tiles == list(TILES) else {}
    assert sum(tiles) == free, (tiles, free)
    offs = [0]
    for w in tiles:
        offs.append(offs[-1] + w)
    nt = len(tiles)
    max_f = max(tiles)
    f32 = mybir.dt.float32
    bf16 = mybir.dt.bfloat16
    Alu = mybir.AluOpType
    Act = mybir.ActivationFunctionType

    nc = bacc.Bacc(
        "TRN2",
        target_bir_lowering=False,
        debug=False,
        enable_asserts=False,
        num_devices=NCORES,
    )
    x_ap = nc.dram_tensor("x", (P, free), f32, kind="ExternalInput").ap()
    prm_ap = nc.dram_tensor("prm", (1, NPRM), f32, kind="ExternalInput").ap()
    nout = NC_CNT * nt + 3 * nt + 2 * NPRM
    out_ap = nc.dram_tensor("out", (P, nout), f32, kind="ExternalOutput").ap()

    act_chain = []  # enforce ACT program order so act-table reloads stay rare

    def chain(inst):
        if act_chain:
            bass._add_dep_helper(inst.ins, act_chain[-1].ins, sync=False,
                                 reason="act table phase order")
        act_chain.append(inst)
        return inst

    with tile.TileContext(nc) as tc, ExitStack() as ctx:
        const_pool = ctx.enter_context(tc.tile_pool(name="const", bufs=1))
        xpool = ctx.enter_context(tc.tile_pool(name="xin", bufs=2))
        mpool = ctx.enter_context(tc.tile_pool(name="m", bufs=3))
        upool = ctx.enter_context(tc.tile_pool(name="u", bufs=1))
        sapool = ctx.enter_context(tc.tile_pool(name="sa", bufs=1))
        jd_pool = ctx.enter_context(tc.tile_pool(name="junkd", bufs=2))
        ja_pool = ctx.enter_context(tc.tile_pool(name="junka", bufs=2))
        acc_pool = ctx.enter_context(tc.tile_pool(name="acc", bufs=1))

        # params -> all partitions
        prm_row = const_pool.tile([1, NPRM], f32, tag="prmrow")
        nc.sync.dma_start(prm_row[:], prm_ap[:, :])
        prm_sb = const_pool.tile([P, NPRM], f32, tag="prmsb")
        nc.gpsimd.partition_broadcast(prm_sb[:], prm_row[:])
        a_ap = prm_sb[:, 0:1]

        # one accumulator arena; subtile dep tracking keeps writes parallel
        nacc = (NC_CNT + 3) * nt
        arena = acc_pool.tile([P, nacc], f32, tag="arena")
        accC = [arena[:, i * nt:(i + 1) * nt] for i in range(NC_CNT)]
        accSP = arena[:, NC_CNT * nt:(NC_CNT + 1) * nt]
        accSG = arena[:, (NC_CNT + 1) * nt:(NC_CNT + 2) * nt]
        accM = arena[:, (NC_CNT + 2) * nt:(NC_CNT + 3) * nt]

        sas = []
        uts = []
        # DVE: per tile [m_t, u_t] hoisted before the previous tile's counts
        # so ACT's sigmoid chain is never starved; ACT: sigmoid/Ln pairwise
        # phases to overlap Ln work with DVE counts (act-table loads ~1.3us).
        def emit_counts(t):
            for i in range(N_DVE_CNT):
                jt = jd_pool.tile([P, tiles[t]], bf16, tag="junkd", name="junkd")
                nc.vector.tensor_scalar(
                    out=jt[:], in0=uts[t][:], scalar1=prm_sb[:, 1 + i:2 + i],
                    scalar2=None, op0=Alu.is_ge, op1=Alu.add,
                    accum_out=accC[i][:, t:t + 1],
                )

        def emit_ln(t):
            chain(nc.scalar.activation(
                out=sas[t][:], in_=sas[t][:], func=Act.Ln,
                accum_out=accSP[:, t:t + 1],
            ))

        ln_done = set()
        for t in range(nt):
            tf = tiles[t]
            xt = xpool.tile([P, tf], f32, tag="xt", name="xt")
            nc.sync.dma_start(xt[:], x_ap[:, offs[t]:offs[t + 1]])

            mt = mpool.tile([P, tf], f32, tag="mt", name="mt")
            nc.vector.tensor_scalar(
                out=mt[:], in0=xt[:], scalar1=a_ap, scalar2=None, op0=Alu.max)

            # u = m - A = relu(x - A) (bf16 for cheap counts); the same op
            # accumulates sum(m - A) in f32; host adds N*A back in f64
            ut = upool.tile([P, tf], bf16, tag=f"ut{t}", name=f"ut{t}")
            nc.vector.tensor_scalar(
                out=ut[:], in0=mt[:], scalar1=a_ap, scalar2=None,
                op0=Alu.subtract, op1=Alu.add,
                accum_out=accM[:, t:t + 1])
            uts.append(ut)

            # sigmoid sum over m; output kept for sp2 = -ln(sigmoid(m))
            sa = sapool.tile([P, tf], f32, tag=f"sa{t}", name=f"sa{t}")
            chain(nc.scalar.activation(
                out=sa[:], in_=mt[:], func=Act.Sigmoid,
                accum_out=accSG[:, t:t + 1],
            ))
            sas.append(sa)

            if t >= 1:
                emit_counts(t - 1)
            for j in ln_plan.get(t, []):
                emit_ln(j)
                ln_done.add(j)
        emit_counts(nt - 1)

        # sigmoid probe while still on the sigmoid table
        psg = const_pool.tile([P, NPRM], f32, tag="psg")
        chain(nc.scalar.activation(out=psg[:], in_=prm_sb[:], func=Act.Sigmoid))

        for t in range(nt):
            if t not in ln_done:
                emit_ln(t)

        # softplus-residual probe via the same sigmoid -> Ln chain
        psp = const_pool.tile([P, NPRM], f32, tag="psp")
        chain(nc.scalar.activation(out=psp[:], in_=psg[:], func=Act.Ln))

        # write outputs
        col = (NC_CNT + 3) * nt
        nc.sync.dma_start(out_ap[:, 0:col], arena[:])
        nc.sync.dma_start(out_ap[:, col:col + NPRM], psp[:])
        col += NPRM
        nc.sync.dma_start(out_ap[:, col:col + NPRM], psg[:])

    nc.compile()
    return nc


def _get_nc():
    if "nc" not in _CACHE:
        _CACHE["nc"] = build_nc()
    return _CACHE["nc"]


def _host_prepass(preds_flat, targs_flat):
    N = preds_flat.size
    pos_mask = targs_flat == 1
    n_pos = int(pos_mask.sum())
    pos_x = preds_flat[pos_mask]
    n_neg = N - n_pos
    if n_pos == 0:
        n_hns = int(DEFAULT_NEG_PERC * n_neg)
    else:
        n_hns = min(n_pos * OHNM_RATIO, n_neg)

    pos64 = pos_x.astype(np.float64)
    S_pos_sp = _np_softplus(pos64).sum()
    S_pos_sg = _np_sigmoid(pos64).sum()
    S_pos_x = pos64.sum()

    # threshold estimate from a subsample of negatives
    sub = preds_flat[::16]
    subn = sub[targs_flat[::16] == 0]
    if n_hns >= n_neg:
        # select-all-negatives: anchor far below the data so m = x exactly
        tau_hat, spread = -40.0, 1.0
    elif n_hns <= 0 or len(subn) < 100:
        tau_hat, spread = 0.0, 1.0
    else:
        q = n_hns / n_neg
        r = max(1, min(int(round(q * len(subn))), len(subn) - 1))
        part = np.partition(subn, len(subn) - r)
        tau_hat = float(part[len(subn) - r])
        h = 0.05
        dens = ((subn > tau_hat - h) & (subn < tau_hat + h)).sum() / (2 * h * len(subn))
        sig = np.sqrt(q * (1 - q) / len(subn)) / max(dens, 1e-9)
        spread = float(max(8 * sig, 0.005))
    return dict(N=N, n_pos=n_pos, n_neg=n_neg, n_hns=n_hns, pos_x=pos_x,
                S_pos_sp=S_pos_sp, S_pos_sg=S_pos_sg, S_pos_x=S_pos_x,
                tau_hat=tau_hat, spread=spread)


def _make_params(tau_hat, spread):
    """A (f32 anchor), u-space count thresholds cs, x-space bounds, prm tensor."""
    rel = np.linspace(-1.0, 1.0, NC_CNT)
    A = np.float32(tau_hat - 1.5 * spread)
    grid = tau_hat + rel * spread
    cs = (grid - float(A)).astype(np.float32)
    assert len(cs) == NC_CNT
    # thresholds counted via ACT Sign must not tie with bf16 u values
    for i in range(N_DVE_CNT, NC_CNT):
        c = np.float32(cs[i])
        if np.float32(ml_dtypes.bfloat16(c)) == c:
            c = np.float32(c + abs(c) * 2.0 ** -10 + 1e-30)
        cs[i] = c
    bounds = np.concatenate([[float(A)], float(A) + cs.astype(np.float64)])
    prm = np.zeros((1, NPRM), np.float32)
    prm[0, 0] = A
    prm[0, 1:1 + NC_CNT] = cs
    prm[0, 9:9 + (NC_CNT - N_DVE_CNT)] = -cs[N_DVE_CNT:]
    # probe slots: the bounds and cell midpoints (x-space)
    mids = 0.5 * (bounds[:-1] + bounds[1:])
    probes = np.concatenate([bounds, mids])  # 8 + 7 = 15 values
    prm[0, 16:16 + len(probes)] = probes.astype(np.float32)
    return A, cs, bounds, prm


def _get_runner():
    """Cached jitted SPMD runner (mirrors bass2jax.run_bass_via_pjrt, but the
    lowered/jitted callable is built once and reused across calls)."""
    if "runner" in _CACHE:
        return _CACHE["runner"]
    import jax
    import numpy as _np
    from jax.sharding import Mesh, PartitionSpec
    from jax.experimental.shard_map import shard_map
    from concourse import mybir
    from concourse.bass2jax import (_bass_exec_p, install_neuronx_cc_hook,
                                    partition_id_tensor)

    install_neuronx_cc_hook()
    nc = _get_nc()
    partition_name = (nc.partition_id_tensor.name
                      if nc.partition_id_tensor else None)

    in_names, out_names, out_avals, zero_outs = [], [], [], []
    for alloc in nc.m.functions[0].allocations:
        if not isinstance(alloc, mybir.MemoryLocationSet):
            continue
        name = alloc.memorylocations[0].name
        if alloc.kind == "ExternalInput":
            if name != partition_name:
                in_names.append(name)
        elif alloc.kind == "ExternalOutput":
            out_names.append(name)
            shape = tuple(alloc.tensor_shape)
            dtype = mybir.dt.np(alloc.dtype)
            out_avals.append(jax.core.ShapedArray(shape, dtype))
            zero_outs.append(_np.zeros(shape, dtype))
    n_params = len(in_names)
    n_outs = len(out_avals)
    all_names = in_names + out_names
    if partition_name is not None:
        all_names = all_names + [partition_name]

    def _body(*args):
        operands = list(args)
        if partition_name is not None:
            operands.append(partition_id_tensor())
        outs = _bass_exec_p.bind(
            *operands,
            out_avals=tuple(out_avals),
            in_names=tuple(all_names),
            out_names=tuple(out_names),
            lowering_input_output_aliases=(),
            sim_require_finite=True,
            sim_require_nnan=True,
            nc=nc,
        )
        return tuple(outs)

    devices = jax.devices()[:NCORES]
    mesh = Mesh(np.asarray(devices), ("core",))
    in_specs = (PartitionSpec("core"),) * (n_params + n_outs)
    out_specs = (PartitionSpec("core"),) * n_outs
    donate = tuple(range(n_params, n_params + n_outs))
    sharded = jax.jit(
        shard_map(_body, mesh=mesh, in_specs=in_specs, out_specs=out_specs,
                  check_rep=False),
        donate_argnums=donate, keep_unused=True,
    )
    _CACHE["runner"] = (sharded, in_names, out_names, zero_outs)
    return _CACHE["runner"]


def _run_device(shards, prm):
    """shards: [8, 128, FREE] f32. Returns list of 8 out arrays [P, NOUT]."""
    sharded, in_names, out_names, zero_outs = _get_runner()
    per_core = [{"x": shards[c], "prm": prm} for c in range(NCORES)]
    concat_in = [np.concatenate([per_core[c][n] for c in range(NCORES)], axis=0)
                 for n in in_names]
    concat_zeros = [np.zeros((NCORES * z.shape[0], *z.shape[1:]), z.dtype)
                    for z in zero_outs]
    out_arrs = sharded(*concat_in, *concat_zeros)
    res = []
    for c in range(NCORES):
        d = {}
        for i, name in enumerate(out_names):
            arr = np.asarray(out_arrs[i])
            rows = arr.shape[0] // NCORES
            d[name] = arr[c * rows:(c + 1) * rows]
        res.append(d)
    _CACHE["last_outs"] = res
    return [r["out"] for r in res]


def _merge(outs, ph, A, cs, bounds, prm):
    """Host-side merge of per-core outputs into the final scalar."""
    N, n_pos, n_hns = ph["N"], ph["n_pos"], ph["n_hns"]
    n_neg = ph["n_neg"]
    tot = np.zeros(NOUT, np.float64)
    for o in outs:
        tot += o.astype(np.float64).sum(axis=0)
    C = np.array([tot[COL_C + i * NT: COL_C + (i + 1) * NT].sum()
                  for i in range(NC_CNT)])
    # ACT-measured counts arrive as sum(sign(u - c)) = 2*C - N
    for i in range(N_DVE_CNT, NC_CNT):
        C[i] = (C[i] + N) / 2.0
    SP2 = -tot[COL_SP:COL_SP + NT].sum()  # sum ln(1+exp(-m)) = -sum ln(sigmoid)
    SG = tot[COL_SG:COL_SG + NT].sum()
    # COL_M holds sum(m - A); reconstruct sum(m) in f64
    SM = tot[COL_M:COL_M + NT].sum() + N * float(A)
    SP = SM + SP2                          # sum softplus(m)

    # device-LUT values at probes (mean across cores/partitions; identical data)
    denom_probe = NCORES * P
    psp = tot[COL_PSP:COL_PSP + NPRM] / denom_probe
    psg = tot[COL_PSG:COL_PSG + NPRM] / denom_probe
    probe_x = prm[0, 16:].astype(np.float64)
    # LUT correction: delta(v) = lut(v) - exact(v), interpolated at probes
    pv = probe_x[:15]
    order = np.argsort(pv)
    pv_s = pv[order]
    dsp_s = (-psp[16:31] - _np_softplus(-pv))[order]
    dsg_s = (psg[16:31] - _np_sigmoid(pv))[order]

    def lut_sp(v):
        # softplus(v) = v + sp2(v); device measures sp2 via exp/ln chain
        return float(v + _np_softplus(-np.asarray(v, np.float64))
                     + np.interp(v, pv_s, dsp_s))

    def lut_sg(v):
        return float(_np_sigmoid(v) + np.interp(v, pv_s, dsg_s))

    # subtract positives' contribution to device stats (host-exact simulation)
    pos32 = ph["pos_x"].astype(np.float32)
    Af = np.float32(A)
    upos = np.maximum(pos32 - Af, 0).astype(ml_dtypes.bfloat16).astype(np.float32)
    Cpos = np.array([(upos >= c).sum() for c in cs], np.float64)
    mpos = np.maximum(pos32, Af).astype(np.float64)      # f32 m path
    SPpos = (mpos + _np_softplus(-mpos) + np.interp(mpos, pv_s, dsp_s)).sum()
    SGpos = (_np_sigmoid(mpos) + np.interp(mpos, pv_s, dsg_s)).sum()

    Cn = C - Cpos
    SPn = SP - SPpos
    SGn = SG - SGpos
    Nn = N - n_pos

    if n_hns <= 0:
        sel_sp = 0.0
        sel_sg = 0.0
    else:
        k = float(n_hns)
        if k > Cn[0] or k < Cn[-1]:
            raise BracketMiss(Cn)
        B = bounds[1:]  # x-space grid points aligned with Cn
        sel_sp = SPn - (Nn - Cn[0]) * lut_sp(float(A))
        sel_sg = SGn - (Nn - Cn[0]) * lut_sg(float(A))
        j = 0
        while j + 1 < NC_CNT and Cn[j + 1] >= k:
            j += 1
        for i in range(j):
            pop = Cn[i] - Cn[i + 1]
            mid = 0.5 * (B[i] + B[i + 1])
            sel_sp -= pop * lut_sp(mid)
            sel_sg -= pop * lut_sg(mid)
        pop_j = Cn[j] - (Cn[j + 1] if j + 1 < NC_CNT else 0.0)
        excl = Cn[j] - k
        hi = B[j + 1] if j + 1 < NC_CNT else B[j] + 1.0
        if excl > 0 and pop_j > 0:
            f = excl / pop_j
            mid = B[j] + 0.5 * f * (hi - B[j])
            sel_sp -= excl * lut_sp(mid)
            sel_sg -= excl * lut_sg(mid)

    inter = ph["S_pos_sg"]
    denom = (sel_sg + ph["S_pos_sg"]) + n_pos
    dice = 1.0 - (2.0 * inter + EPS) / (denom + EPS)
    bce = (sel_sp + (ph["S_pos_sp"] - ph["S_pos_x"])) / (n_hns + n_pos)
    return np.float32(dice + bce)


class BracketMiss(RuntimeError):
    def __init__(self, counts):
        super().__init__(f"bracket miss: counts={counts}")
        self.counts = counts


def kernel(preds, targs):
    preds_flat = np.asarray(preds, np.float32).ravel()
    targs_flat = np.asarray(targs).ravel()
    ph = _host_prepass(preds_flat, targs_flat)

    shards = preds_flat.reshape(NCORES, P, FREE)

    tau_hat, spread = ph["tau_hat"], ph["spread"]
    for attempt in range(4):
        A, cs, bounds, prm = _make_params(tau_hat, spread)
        outs = _run_device(shards, prm)
        try:
            return _merge(outs, ph, A, cs, bounds, prm)
        except BracketMiss as bm:
            # widen and recenter using the measured counts, then relaunch
            Cn = bm.counts
            k = ph["n_hns"]
            if k > Cn[0]:
                tau_hat = float(A) - 2.0 * spread
            else:
                tau_hat = float(bounds[-1]) + 2.0 * spread
            spread *= 4.0
    raise RuntimeError("failed to bracket top-k threshold after 4 attempts")


if __name__ == "__main__":
    # quick self-test against numpy ground truth (no jax needed)
    rng = np.random.default_rng(0)
    preds = rng.standard_normal((1, 1, 256, 256, 256), np.float32)
    targs = (rng.random((1, 1, 256, 256, 256)) < 1e-3).astype(np.int32)
    out = kernel(preds, targs)
    print("kernel out:", out)


# revision 30
# speedup vs baseline: 1.0462x; 1.0060x over previous
"""DiceBCE + online-hard-negative-mining loss on 8 Trainium2 NeuronCores.

Key fact: the loss needs only SUMS over the top-n_hns set, never indices, and
BCE loss is monotone in the logit -- so top-k-by-loss == top-k-by-x and the
whole problem reduces to a threshold selection plus masked reductions.

Single streaming pass over preds on device (targs never leaves the host):
 - Host: exact positive-voxel stats in f64 (positives are ~0.1%), n_hns, and
   a subsample-quantile estimate tau_hat of the selection threshold.
 - Device (per core, 1/8 shard of preds; anchor A = tau_hat - 1.5*spread is a
   runtime input, so re-launches never recompile):
     m  = max(x, A)            f32  (DVE tensor_scalar)
     u  = m - A = relu(x - A)  bf16, same op accumulates sum(m - A) in f32
     exact counts #{u >= c_i} at 2 grid thresholds (DVE tensor_scalar+accum)
     sum sigmoid(m)            (ACT, f32-grade; bf16 ACT inputs are biased)
     sum ln(sigmoid(m))        (ACT Ln over the saved sigmoid tile)
       => sum softplus(m) = sum(m) + (-sum ln sigmoid(m)), with both LUTs
          evaluated only on narrow, accurate argument ranges
     LUT probe outputs at the grid/mid points for host-side bias calibration
 - Host merge (f64): threshold selection from exact counts with fractional
   boundary-cell interpolation; below-grid mass is valued at softplus(A)
   (dominated by the x<=A spike sitting exactly at A); positives' device
   contributions subtracted via exact bit-level simulation; dice + BCE
   assembled and cast to f32. A bracket miss re-launches with a wider grid
   (runtime params only).
"""

import os
import numpy as np
import ml_dtypes

OHNM_RATIO = 30
DEFAULT_NEG_PERC = 0.1
EPS = 1e-10

NCORES = 8
P = 128
FREE = 16384          # per-core shard = [128, 16384] = 2,097,152 elements
TILE_F = 4096
TILES = [512, 4096, 4096, 4096, 3584]    # per-tile free dims (sum = FREE)
NT = len(TILES)
# ACT chain plan: after sig_j (key), emit Ln passes for tiles in value
LN_PLAN = {1: [0, 1]}
NC_CNT = 2            # count thresholds (real grid points; no u>0 count)
NPRM = 32             # param tensor: [0]=A, [1..8]=count thresholds (u-space),
                      # [16..31] = probe values (x-space, sp/sg LUT readback)
TINY = np.float32(1e-35)

N_DVE_CNT = 2                  # count thresholds measured on DVE (is_ge)
N_ACT_CNT = NC_CNT - N_DVE_CNT  # count thresholds measured on ACT (Sign)

# output column layout
COL_C = 0                      # counts: NC_CNT * NT (DVE is_ge first, then ACT sign-sums)
COL_SP = COL_C + NC_CNT * NT   # sum sp2(m) = ln(1+exp(-m)): NT
COL_SG = COL_SP + NT           # sum sigmoid(m): NT
COL_M = COL_SG + NT            # sum m (exact f32): NT
COL_PSP = COL_M + NT           # probe sp2: NPRM
COL_PSG = COL_PSP + NPRM       # probe sigmoid: NPRM
NOUT = COL_PSG + NPRM

_CACHE = {}


def _np_softplus(x):
    x = np.asarray(x, np.float64)
    return np.maximum(x, 0) + np.log1p(np.exp(-np.abs(x)))


def _np_sigmoid(x):
    x = np.asarray(x, np.float64)
    return 0.5 * (1 + np.tanh(x / 2))


def build_nc(free=FREE, tile_f=None, tiles=None, ln_plan=None):
    """Build the Bass module (one NeuronCore program, run SPMD on 8 cores)."""
    from contextlib import ExitStack
    import concourse.bass as bass
    import concourse.tile as tile
    from concourse import bacc, mybir

    if tiles is None:
        tiles = [tile_f] * (free // tile_f) if tile_f else list(TILES)
    if ln_plan is None:
        ln_plan = LN_PLAN if tiles == list(TILES) else {}
    assert sum(tiles) == free, (tiles, free)
    offs = [0]
    for w in tiles:
        offs.append(offs[-1] + w)
    nt = len(tiles)
    max_f = max(tiles)
    f32 = mybir.dt.float32
    bf16 = mybir.dt.bfloat16
    Alu = mybir.AluOpType
    Act = mybir.ActivationFunctionType

    nc = bacc.Bacc(
        "TRN2",
        target_bir_lowering=False,
        debug=False,
        enable_asserts=False,
        num_devices=NCORES,
    )
    x_ap = nc.dram_tensor("x", (P, free), f32, kind="ExternalInput").ap()
    prm_ap = nc.dram_tensor("prm", (1, NPRM), f32, kind="ExternalInput").ap()
    nout = NC_CNT * nt + 3 * nt + 2 * NPRM
    out_ap = nc.dram_tensor("out", (P, nout), f32, kind="ExternalOutput").ap()

    act_chain = []  # enforce ACT program order so act-table reloads stay rare

    def chain(inst):
        if act_chain:
            bass._add_dep_helper(inst.ins, act_chain[-1].ins, sync=False,
                                 reason="act table phase order")
        act_chain.append(inst)
        return inst

    with tile.TileContext(nc) as tc, ExitStack() as ctx:
        const_pool = ctx.enter_context(tc.tile_pool(name="const", bufs=1))
        xpool = ctx.enter_context(tc.tile_pool(name="xin", bufs=2))
        mpool = ctx.enter_context(tc.tile_pool(name="m", bufs=3))
        upool = ctx.enter_context(tc.tile_pool(name="u", bufs=1))
        sapool = ctx.enter_context(tc.tile_pool(name="sa", bufs=1))
        jd_pool = ctx.enter_context(tc.tile_pool(name="junkd", bufs=2))
        ja_pool = ctx.enter_context(tc.tile_pool(name="junka", bufs=2))
        acc_pool = ctx.enter_context(tc.tile_pool(name="acc", bufs=1))

        # params -> all partitions
        prm_row = const_pool.tile([1, NPRM], f32, tag="prmrow")
        nc.sync.dma_start(prm_row[:], prm_ap[:, :])
        prm_sb = const_pool.tile([P, NPRM], f32, tag="prmsb")
        nc.gpsimd.partition_broadcast(prm_sb[:], prm_row[:])
        a_ap = prm_sb[:, 0:1]

        # one accumulator arena; subtile dep tracking keeps writes parallel
        nacc = (NC_CNT + 3) * nt
        arena = acc_pool.tile([P, nacc], f32, tag="arena")
        accC = [arena[:, i * nt:(i + 1) * nt] for i in range(NC_CNT)]
        accSP = arena[:, NC_CNT * nt:(NC_CNT + 1) * nt]
        accSG = arena[:, (NC_CNT + 1) * nt:(NC_CNT + 2) * nt]
        accM = arena[:, (NC_CNT + 2) * nt:(NC_CNT + 3) * nt]

        sas = []
        uts = []
        # DVE: per tile [m_t, u_t] hoisted before the previous tile's counts
        # so ACT's sigmoid chain is never starved; ACT: sigmoid/Ln pairwise
        # phases to overlap Ln work with DVE counts (act-table loads ~1.3us).
        def emit_counts(t):
            for i in range(N_DVE_CNT):
                jt = jd_pool.tile([P, tiles[t]], bf16, tag="junkd", name="junkd")
                nc.vector.tensor_scalar(
                    out=jt[:], in0=uts[t][:], scalar1=prm_sb[:, 1 + i:2 + i],
                    scalar2=None, op0=Alu.is_ge, op1=Alu.add,
                    accum_out=accC[i][:, t:t + 1],
                )

        def emit_ln(t):
            chain(nc.scalar.activation(
                out=sas[t][:], in_=sas[t][:], func=Act.Ln,
                accum_out=accSP[:, t:t + 1],
            ))

        ln_done = set()
        for t in range(nt):
            tf = tiles[t]
            xt = xpool.tile([P, tf], f32, tag="xt", name="xt")
            nc.sync.dma_start(xt[:], x_ap[:, offs[t]:offs[t + 1]])

            mt = mpool.tile([P, tf], f32, tag="mt", name="mt")
            nc.vector.tensor_scalar(
                out=mt[:], in0=xt[:], scalar1=a_ap, scalar2=None, op0=Alu.max)

            # u = m - A = relu(x - A) (bf16 for cheap counts); the same op
            # accumulates sum(m - A) in f32; host adds N*A back in f64
            ut = upool.tile([P, tf], bf16, tag=f"ut{t}", name=f"ut{t}")
            nc.vector.tensor_scalar(
                out=ut[:], in0=mt[:], scalar1=a_ap, scalar2=None,
                op0=Alu.subtract, op1=Alu.add,
                accum_out=accM[:, t:t + 1])
            uts.append(ut)

            # sigmoid sum over m; output kept for sp2 = -ln(sigmoid(m))
            sa = sapool.tile([P, tf], f32, tag=f"sa{t}", name=f"sa{t}")
            chain(nc.scalar.activation(
                out=sa[:], in_=mt[:], func=Act.Sigmoid,
                accum_out=accSG[:, t:t + 1],
            ))
            sas.append(sa)

            if t >= 1:
                emit_counts(t - 1)
            for j in ln_plan.get(t, []):
                emit_ln(j)
                ln_done.add(j)
        emit_counts(nt - 1)

        # sigmoid probe while still on the sigmoid table
        psg = const_pool.tile([P, NPRM], f32, tag="psg")
        chain(nc.scalar.activation(out=psg[:], in_=prm_sb[:], func=Act.Sigmoid))

        for t in range(nt):
            if t not in ln_done:
                emit_ln(t)

        # softplus-residual probe via the same sigmoid -> Ln chain
        psp = const_pool.tile([P, NPRM], f32, tag="psp")
        chain(nc.scalar.activation(out=psp[:], in_=psg[:], func=Act.Ln))

        # write outputs
        col = (NC_CNT + 3) * nt
        nc.sync.dma_start(out_ap[:, 0:col], arena[:])
        nc.sync.dma_start(out_ap[:, col:col + NPRM], psp[:])
        col += NPRM
        nc.sync.dma_start(out_ap[:, col:col + NPRM], psg[:])

    nc.compile()
    return nc


def _get_nc():
    if "nc" not in _CACHE:
        _CACHE["nc"] = build_nc()
    return _CACHE["nc"]


def _host_prepass(preds_flat, targs_flat):
    N = preds_flat.size
    pos_mask = targs_flat == 1
    n_pos = int(pos_mask.sum())
    pos_x = preds_flat[pos_mask]
    n_neg = N - n_pos
    if n_pos == 0:
        n_hns = int(DEFAULT_NEG_PERC * n_neg)
    else:
        n_hns = min(n_pos * OHNM_RATIO, n_neg)

    pos64 = pos_x.astype(np.float64)
    S_pos_sp = _np_softplus(pos64).sum()
    S_pos_sg = _np_sigmoid(pos64).sum()
    S_pos_x = pos64.sum()

    # threshold estimate from a subsample of negatives
    sub = preds_flat[::16]
    subn = sub[targs_flat[::16] == 0]
    if n_hns >= n_neg:
        # select-all-negatives: anchor far below the data so m = x exactly
        tau_hat, spread = -40.0, 1.0
    elif n_hns <= 0 or len(subn) < 100:
        tau_hat, spread = 0.0, 1.0
    else:
        q = n_hns / n_neg
        r = max(1, min(int(round(q * len(subn))), len(subn) - 1))
        part = np.partition(subn, len(subn) - r)
        tau_hat = float(part[len(subn) - r])
        h = 0.05
        dens = ((subn > tau_hat - h) & (subn < tau_hat + h)).sum() / (2 * h * len(subn))
        sig = np.sqrt(q * (1 - q) / len(subn)) / max(dens, 1e-9)
        spread = float(max(8 * sig, 0.005))
    return dict(N=N, n_pos=n_pos, n_neg=n_neg, n_hns=n_hns, pos_x=pos_x,
                S_pos_sp=S_pos_sp, S_pos_sg=S_pos_sg, S_pos_x=S_pos_x,
                tau_hat=tau_hat, spread=spread)


def _make_params(tau_hat, spread):
    """A (f32 anchor), u-space count thresholds cs, x-space bounds, prm tensor."""
    rel = np.linspace(-1.0, 1.0, NC_CNT)
    A = np.float32(tau_hat - 1.5 * spread)
    grid = tau_hat + rel * spread
    cs = (grid - float(A)).astype(np.float32)
    assert len(cs) == NC_CNT
    # thresholds counted via ACT Sign must not tie with bf16 u values
    for i in range(N_DVE_CNT, NC_CNT):
        c = np.float32(cs[i])
        if np.float32(ml_dtypes.bfloat16(c)) == c:
            c = np.float32(c + abs(c) * 2.0 ** -10 + 1e-30)
        cs[i] = c
    bounds = np.concatenate([[float(A)], float(A) + cs.astype(np.float64)])
    prm = np.zeros((1, NPRM), np.float32)
    prm[0, 0] = A
    prm[0, 1:1 + NC_CNT] = cs
    prm[0, 9:9 + (NC_CNT - N_DVE_CNT)] = -cs[N_DVE_CNT:]
    # probe slots: the bounds and cell midpoints (x-space)
    mids = 0.5 * (bounds[:-1] + bounds[1:])
    probes = np.concatenate([bounds, mids])  # 8 + 7 = 15 values
    prm[0, 16:16 + len(probes)] = probes.astype(np.float32)
    return A, cs, bounds, prm


def _get_runner():
    """Cached jitted SPMD runner (mirrors bass2jax.run_bass_via_pjrt, but the
    lowered/jitted callable is built once and reused across calls)."""
    if "runner" in _CACHE:
        return _CACHE["runner"]
    import jax
    import numpy as _np
    from jax.sharding import Mesh, PartitionSpec
    from jax.experimental.shard_map import shard_map
    from concourse import mybir
    from concourse.bass2jax import (_bass_exec_p, install_neuronx_cc_hook,
                                    partition_id_tensor)

    install_neuronx_cc_hook()
    nc = _get_nc()
    partition_name = (nc.partition_id_tensor.name
                      if nc.partition_id_tensor else None)

    in_names, out_names, out_avals, zero_outs = [], [], [], []
    for alloc in nc.m.functions[0].allocations:
        if not isinstance(alloc, mybir.MemoryLocationSet):
            continue
        name = alloc.memorylocations[0].name
        if alloc.kind == "ExternalInput":
            if name != partition_name:
                in_names.append(name)
        elif alloc.kind == "ExternalOutput":
            out_names.append(name)
            shape = tuple(alloc.tensor_shape)
            dtype = mybir.dt.np(alloc.dtype)
            out_avals.append(jax.core.ShapedArray(shape, dtype))
            zero_outs.append(_np.zeros(shape, dtype))
    n_params = len(in_names)
    n_outs = len(out_avals)
    all_names = in_names + out_names
    if partition_name is not None:
        all_names = all_names + [partition_name]

    def _body(*args):
        operands = list(args)
        if partition_name is not None:
            operands.append(partition_id_tensor())
        outs = _bass_exec_p.bind(
            *operands,
            out_avals=tuple(out_avals),
            in_names=tuple(all_names),
            out_names=tuple(out_names),
            lowering_input_output_aliases=(),
            sim_require_finite=True,
            sim_require_nnan=True,
            nc=nc,
        )
        return tuple(outs)

    devices = jax.devices()[:NCORES]
    mesh = Mesh(np.asarray(devices), ("core",))
    in_specs = (PartitionSpec("core"),) * (n_params + n_outs)
    out_specs = (PartitionSpec("core"),) * n_outs
    donate = tuple(range(n_params, n_params + n_outs))
    sharded = jax.jit(
        shard_map(_body, mesh=mesh, in_specs=in_specs, out_specs=out_specs,
                  check_rep=False),
        donate_argnums=donate, keep_unused=True,
    )
    _CACHE["runner"] = (sharded, in_names, out_names, zero_outs)
    return _CACHE["runner"]


def _run_device(shards, prm):
    """shards: [8, 128, FREE] f32. Returns list of 8 out arrays [P, NOUT]."""
    sharded, in_names, out_names, zero_outs = _get_runner()
    per_core = [{"x": shards[c], "prm": prm} for c in range(NCORES)]
    concat_in = [np.concatenate([per_core[c][n] for c in range(NCORES)], axis=0)
                 for n in in_names]
    concat_zeros = [np.zeros((NCORES * z.shape[0], *z.shape[1:]), z.dtype)
                    for z in zero_outs]
    out_arrs = sharded(*concat_in, *concat_zeros)
    res = []
    for c in range(NCORES):
        d = {}
        for i, name in enumerate(out_names):
            arr = np.asarray(out_arrs[i])
            rows = arr.shape[0] // NCORES
            d[name] = arr[c * rows:(c + 1) * rows]
        res.append(d)
    _CACHE["last_outs"] = res
    return [r["out"] for r in res]


def _merge(outs, ph, A, cs, bounds, prm):
    """Host-side merge of per-core outputs into the final scalar."""
    N, n_pos, n_hns = ph["N"], ph["n_pos"], ph["n_hns"]
    n_neg = ph["n_neg"]
    tot = np.zeros(NOUT, np.float64)
    for o in outs:
        tot += o.astype(np.float64).sum(axis=0)
    C = np.array([tot[COL_C + i * NT: COL_C + (i + 1) * NT].sum()
                  for i in range(NC_CNT)])
    # ACT-measured counts arrive as sum(sign(u - c)) = 2*C - N
    for i in range(N_DVE_CNT, NC_CNT):
        C[i] = (C[i] + N) / 2.0
    SP2 = -tot[COL_SP:COL_SP + NT].sum()  # sum ln(1+exp(-m)) = -sum ln(sigmoid)
    SG = tot[COL_SG:COL_SG + NT].sum()
    # COL_M holds sum(m - A); reconstruct sum(m) in f64
    SM = tot[COL_M:COL_M + NT].sum() + N * float(A)
    SP = SM + SP2                          # sum softplus(m)

    # device-LUT values at probes (mean across cores/partitions; identical data)
    denom_probe = NCORES * P
    psp = tot[COL_PSP:COL_PSP + NPRM] / denom_probe
    psg = tot[COL_PSG:COL_PSG + NPRM] / denom_probe
    probe_x = prm[0, 16:].astype(np.float64)
    # LUT correction: delta(v) = lut(v) - exact(v), interpolated at probes
    pv = probe_x[:15]
    order = np.argsort(pv)
    pv_s = pv[order]
    dsp_s = (-psp[16:31] - _np_softplus(-pv))[order]
    dsg_s = (psg[16:31] - _np_sigmoid(pv))[order]

    def lut_sp(v):
        # softplus(v) = v + sp2(v); device measures sp2 via exp/ln chain
        return float(v + _np_softplus(-np.asarray(v, np.float64))
                     + np.interp(v, pv_s, dsp_s))

    def lut_sg(v):
        return float(_np_sigmoid(v) + np.interp(v, pv_s, dsg_s))

    # subtract positives' contribution to device stats (host-exact simulation)
    pos32 = ph["pos_x"].astype(np.float32)
    Af = np.float32(A)
    upos = np.maximum(pos32 - Af, 0).astype(ml_dtypes.bfloat16).astype(np.float32)
    Cpos = np.array([(upos >= c).sum() for c in cs], np.float64)
    mpos = np.maximum(pos32, Af).astype(np.float64)      # f32 m path
    SPpos = (mpos + _np_softplus(-mpos) + np.interp(mpos, pv_s, dsp_s)).sum()
    SGpos = (_np_sigmoid(mpos) + np.interp(mpos, pv_s, dsg_s)).sum()

    Cn = C - Cpos
    SPn = SP - SPpos
    SGn = SG - SGpos
    Nn = N - n_pos

    if n_hns <= 0:
        sel_sp = 0.0
        sel_sg = 0.0
    else:
        k = float(n_hns)
        if k > Cn[0] or k < Cn[-1]:
            raise BracketMiss(Cn)
        B = bounds[1:]  # x-space grid points aligned with Cn
        sel_sp = SPn - (Nn - Cn[0]) * lut_sp(float(A))
        sel_sg = SGn - (Nn - Cn[0]) * lut_sg(float(A))
        j = 0
        while j + 1 < NC_CNT and Cn[j + 1] >= k:
            j += 1
        for i in range(j):
            pop = Cn[i] - Cn[i + 1]
            mid = 0.5 * (B[i] + B[i + 1])
            sel_sp -= pop * lut_sp(mid)
            sel_sg -= pop * lut_sg(mid)
        pop_j = Cn[j] - (Cn[j + 1] if j + 1 < NC_CNT else 0.0)
        excl = Cn[j] - k
        hi = B[j + 1] if j + 1 < NC_CNT else B[j] + 1.0
        if excl > 0 and pop_j > 0:
            f = excl / pop_j
            mid = B[j] + 0.5 * f * (hi - B[j])
            sel_sp -= excl * lut_sp(mid)
            sel_sg -= excl * lut_sg(mid)

    inter = ph["S_pos_sg"]
    denom = (sel_sg + ph["S_pos_sg"]) + n_pos
    dice = 1.0 - (2.0 * inter + EPS) / (denom + EPS)
    bce = (sel_sp + (ph["S_pos_sp"] - ph["S_pos_x"])) / (n_hns + n_pos)
    return np.float32(dice + bce)


class BracketMiss(RuntimeError):
    def __init__(self, counts):
        super().__init__(f"bracket miss: counts={counts}")
        self.counts = counts


def kernel(preds, targs):
    preds_flat = np.asarray(preds, np.float32).ravel()
    targs_flat = np.asarray(targs).ravel()
    ph = _host_prepass(preds_flat, targs_flat)

    shards = preds_flat.reshape(NCORES, P, FREE)

    tau_hat, spread = ph["tau_hat"], ph["spread"]
    for attempt in range(4):
        A, cs, bounds, prm = _make_params(tau_hat, spread)
        outs = _run_device(shards, prm)
        try:
            return _merge(outs, ph, A, cs, bounds, prm)
        except BracketMiss as bm:
            # widen and recenter using the measured counts, then relaunch
            Cn = bm.counts
            k = ph["n_hns"]
            if k > Cn[0]:
                tau_hat = float(A) - 2.0 * spread
            else:
                tau_hat = float(bounds[-1]) + 2.0 * spread
            spread *= 4.0
    raise RuntimeError("failed to bracket top-k threshold after 4 attempts")


if __name__ == "__main__":
    # quick self-test against numpy ground truth (no jax needed)
    rng = np.random.default_rng(0)
    preds = rng.standard_normal((1, 1, 256, 256, 256), np.float32)
    targs = (rng.random((1, 1, 256, 256, 256)) < 1e-3).astype(np.int32)
    out = kernel(preds, targs)
    print("kernel out:", out)
